# revision 42
# baseline (speedup 1.0000x reference)
# Block-diagonal masked SDPA (Qwen2.5-VL vision style) for Trainium2.
#
# Full inputs:  q/k/v [1, 16, 4096, 80] f32, cu_seqlens [9] i32, scaling f32.
# Output:       [1, 4096, 16, 80] f32.
#
# Sharding: tensor-parallel over heads — 2 heads per core on 8 cores; each
# core computes its heads' full masked SDPA independently (no collectives).
#
# Strategy (host-specialized on cu_seqlens, same program on all cores):
#   Work is decomposed per SEGMENT, with k-chunks of 128 keys aligned to the
#   segment start, so no mask is ever needed: the last chunk of a segment
#   simply uses pn < 128 partitions.  V is host-packed segment-aligned as
#   [128, NCH, 81] bf16 (81st column = ones for the softmax denominator;
#   padding rows zero).  Everything runs as single bf16 matmuls: the 2e-2
#   harness gate leaves bf16 (~3e-3) ample margin.
#
#   Per segment, q is split into jobs of <= 512 columns.  Per chunk:
#     S^T [pn, qn] = K_chunk^T Q_job      (1 bf16 matmul, f32 PSUM)
#     P = exp(S^T) -> bf16 SBUF           (ACT exp, or DVE/Pool via a
#                                          Schraudolph bit-trick exp)
#     ot [81, qn] += V_chunk^T P          (1 bf16 matmul, V stationary)
#   Epilogue per job: evacuate ot PSUM->SBUF into half of a pair tile; one
#   DMA per job-PAIR ships the raw [81, .] numerator+denominator slab to a
#   job-major DRAM layout from the otherwise idle SP queue.  The gather
#   step on the host performs the final divide-by-denominator and [d, q] ->
#   [q, d] layout transpose.
#
# Trace-driven pipeline optimizations (v2):
#   - K and Q are host-packed into ONE DRAM tensor per head ([D, 2, S]);
#     input DMAs are issued K-of-first-segment first so the PE never waits,
#     and half as many DMA-issue instructions sit on the SP queue.
#   - The PE p-state ramps 0.65 -> 1.2 -> 2.4 GHz over ~3us of continuous
#     execution; WARMUP dummy matmuls during the DMA prologue pre-ramp the
#     clock so real matmuls start at speed.
#   - exp / copies are greedily balanced across THREE engines (ACT, DVE,
#     Pool); output DMAs moved off Pool (gpsimd) onto the idle SP queue.
#
# Exp instructions are widened (two chunks share one PSUM st tile and one
# exp) to amortize the ~200ns/instr access-latency bubble.  PSUM
# accumulation groups are bank-granular (2KB zero region): same-bank chunk
# pairs accumulate under one start/stop.

import os

import numpy as np

S = 4096
H = 16
D = 80
P = 128
N_CORES = 8
HPC = H // N_CORES  # heads per core

# Engine-balance cost model (ns) for ACT vs DVE vs Pool assignment.
ACT_COL = 1.0 / 1.2
ACT_FIX = 260.0
DVE_COL = 1.0 / 0.96
DVE_FIX = 155.0
POOL_COL = 1.0 / 0.5
POOL_FIX = 260.0

# Schraudolph exp on DVE/Pool: bf16(e^x) bit pattern ~= u16(x * 184.665 + B).
# +0.5 centers the f32->i16 truncation into round-to-nearest.
SCHRAUD_A = 128.0 / float(np.log(2.0))
SCHRAUD_B = 16250.5 + 0.5
# Only segments this long get Schraudolph exp: short segments have large
# softmax weights, amplifying the ~3% Schraudolph error in absolute terms.
DVE_MIN_L = 400

DVE_EXP = os.environ.get("KERNEL_DVE_EXP", "1") == "1"  # offload exp to DVE
# NOTE: GPSIMD/Pool cannot access PSUM (BIR verifier), so it cannot join the
# exp/copy balance (both read PSUM); it stays free for SBUF-side work only.
POOL_EXP = os.environ.get("KERNEL_POOL", "0") == "1"
WARMUP = int(os.environ.get("KERNEL_WARMUP", "8"))  # PE p-state pre-ramp
# AV groups stay pending while this many newer QK groups issue (possibly from
# the NEXT job/head), hiding the last-group exp latency at job boundaries.
AV_LAG = int(os.environ.get("KERNEL_AV_LAG", "3"))
# Job width in q columns.  1024 halves the exp instruction count (one exp
# per chunk covers the whole job width from a 2-bank st tile) — the per-
# instruction ~200-260ns ACT/DVE fixed cost is the binding resource once
# the matmul pipeline is clean.  512 gives narrower exps but 6 st tiles of
# runway instead of 3.
QN = int(os.environ.get("KERNEL_QN", "512"))
# Width of the very first job: narrower = smaller first DMA = earlier start.
FIRST_QN = int(os.environ.get("KERNEL_FIRST_QN", "512"))

_nc_cache = {}
LAST_RESULTS = None  # BassKernelResults of the most recent run (for test.py)


def _segments(cu):
    """[(k0, L, cb, nch)] per segment + total chunk count NCH."""
    segs = []
    cb = 0
    for s in range(len(cu) - 1):
        k0, k1 = int(cu[s]), int(cu[s + 1])
        L = k1 - k0
        if L == 0:
            continue
        nch = -(-L // P)
        segs.append((k0, L, cb, nch))
        cb += nch
    return segs, cb


def _plan(segs):
    """seg_order (largest first) and the global job list in processing order.

    jobs: [(k0, L, cb, nch, qg, qn, out_off)] — out_off is the column offset
    of this job's slab in the job-major DRAM output layout.

    Short segments (L < 512) are exp-bound (the PE finishes their few
    matmuls long before ACT/DVE finish their exps), so their jobs are
    interleaved between long-segment jobs instead of clustered at the end:
    the PE always has long-segment matmuls while a short job's exps drain.
    """
    seg_order = sorted(segs, key=lambda s: -s[1])
    big, small = [], []
    for idx, (k0, L, cb, nch) in enumerate(seg_order):
        o = 0
        while o < L:
            # the very first job stays narrow so it depends only on the
            # small first packed K+Q DMA
            cap = FIRST_QN if (idx == 0 and o == 0) else QN
            qn = min(cap, L - o)
            (big if L >= 512 else small).append((k0, L, cb, nch, k0 + o, qn))
            o += qn
    merged = big[:3]
    bi, si = 3, 0
    while bi < len(big):
        if si < len(small) - 1:  # keep the smallest job for the very end
            merged.append(small[si])
            si += 1
        merged.append(big[bi])
        bi += 1
    merged.extend(small[si:])
    if not big:
        merged = small
    jobs = []
    off = 0
    for k0, L, cb, nch, qg, qn in merged:
        jobs.append((k0, L, cb, nch, qg, qn, off))
        off += qn
    return seg_order, jobs


def _build_nc(cu_tuple):
    from contextlib import ExitStack

    import concourse.bass as bass  # noqa: F401
    import concourse.mybir as mybir
    import concourse.tile as tile
    from concourse import bacc

    f32 = mybir.dt.float32
    bf16 = mybir.dt.bfloat16
    i16 = mybir.dt.int16
    EXP = mybir.ActivationFunctionType.Exp
    MUL = mybir.AluOpType.mult
    ADD = mybir.AluOpType.add

    STW = 1024 if QN > 512 else 512
    ST_BUFS = 3 if QN > 512 else int(os.environ.get("KERNEL_ST_BUFS", "6"))
    OT_BUFS = 2 if QN > 512 else int(os.environ.get("KERNEL_OT_BUFS", "2"))
    cu = np.asarray(cu_tuple, dtype=np.int64)
    segs, NCH = _segments(cu)
    seg_order, jobs = _plan(segs)

    nc = bacc.Bacc(
        "TRN2",
        target_bir_lowering=False,
        debug=False,
        enable_asserts=False,
        num_devices=N_CORES,
    )

    # K at [:, 0, :], Q at [:, 1, :] (Q pre-scaled on host)
    kq_d = nc.dram_tensor("kq", [HPC, D, 2, S], bf16, kind="ExternalInput").ap()
    vh_d = nc.dram_tensor("vh", [HPC, P, NCH, D + 1], bf16, kind="ExternalInput").ap()
    # raw S^T-layout output slabs, job-major: numerators rows 0..79, denom 80
    out_d = nc.dram_tensor("out", [D + 1, HPC * S], f32, kind="ExternalOutput").ap()

    # Greedy ACT/DVE/Pool balance state (build-time, deterministic).
    t_eng = {"act": 0.0, "dve": 0.0, "pool": 0.0}

    def balance(cols, ops, allowed):
        """ops: {name: (col_cost, fix_cost, thunk)}; run on earliest-finish."""
        best, best_c = None, None
        for name in allowed:
            col_c, fix_c, op = ops[name]
            c = t_eng[name] + cols * col_c + fix_c
            if best_c is None or c < best_c:
                best, best_c = name, c
        t_eng[best] = best_c
        ops[best][2]()

    with ExitStack() as ctx:
        tc = ctx.enter_context(tile.TileContext(nc))
        io = ctx.enter_context(tc.tile_pool(name="io", bufs=2))
        wpool = ctx.enter_context(tc.tile_pool(name="wp", bufs=1))
        stpool = ctx.enter_context(
            tc.tile_pool(name="st", bufs=ST_BUFS, space="PSUM")
        )
        otpool = ctx.enter_context(tc.tile_pool(name="ot", bufs=OT_BUFS, space="PSUM"))
        ptpool = ctx.enter_context(tc.tile_pool(name="ptp", bufs=6))
        epool = ctx.enter_context(tc.tile_pool(name="ep", bufs=4))

        # --- PE p-state warmup: dummy matmuls during the DMA prologue ---
        # Reads uninitialized SBUF (harmless: output never read) so it has NO
        # dependencies and starts the instant the preamble barrier drops.
        # Rotates over 4 distinct PSUM regions so Tile's write-write tracking
        # doesn't serialize consecutive dummies.
        if WARMUP:
            wt = wpool.tile([P, 512], bf16, name="wt", tag="wt")
            nc.gpsimd.memset(wt[:], 0)
            wst = [
                stpool.tile([P, STW], f32, name="st", tag="st")
                for _ in range(2 if STW > 512 else 4)
            ]
            for i in range(WARMUP):
                if STW > 512:
                    w, half = wst[(i // 2) % 2], (i % 2) * 512
                else:
                    w, half = wst[i % 4], 0
                nc.tensor.matmul(
                    w[0:64, half : half + 512], lhsT=wt[:, 0:64], rhs=wt[:, 0:512],
                    start=True, stop=True,
                )

        # First processed segment's K loads first (full segment), then its
        # first q job, then V, then the rest — so the first QK matmuls of
        # every job are never waiting on DMA.
        s0 = seg_order[0][0]
        s1 = s0 + seg_order[0][1]
        qn1 = min(FIRST_QN, seg_order[0][1])
        rest = []
        if s0 > 0:
            rest.append(slice(0, s0))
        if s1 < S:
            rest.append(slice(s1, S))

        cb0, nch0 = seg_order[0][2], seg_order[0][3]
        sm1 = s0 + qn1  # first (narrow) job of the first segment
        sm2 = min(s0 + 1024, s1)

        tiles = {}
        for h in range(HPC):
            t = {}
            t["kq"] = io.tile([D, 2, S], bf16, name="kq_s", tag="kq")
            t["v"] = io.tile([P, NCH, D + 1], bf16, name="v_s", tag="v")
            # a small packed DMA covering exactly the first job's K and Q,
            # then a second packed slice (next jobs' Q + next K chunks),
            # then the rest — the first QK matmuls wait only on transfer 1
            # kq slices issue on the SP queue while ALL V slices issue in
            # parallel on the (otherwise idle) GpSimd queue — the first V
            # chunks land in time for the first job's AVs without waiting
            # behind the kq issue ladder.
            nc.sync.dma_start(t["kq"][:, :, s0:sm1], kq_d[h][:, :, s0:sm1])
            nc.gpsimd.dma_start(
                t["v"][:, cb0 : cb0 + nch0], vh_d[h][:, cb0 : cb0 + nch0]
            )
            if sm1 < sm2:
                nc.sync.dma_start(t["kq"][:, :, sm1:sm2], kq_d[h][:, :, sm1:sm2])
            if sm2 < s1:
                nc.sync.dma_start(t["kq"][:, 0, sm2:s1], kq_d[h][:, 0, sm2:s1])
            if sm2 < s1:
                nc.sync.dma_start(t["kq"][:, 1, sm2:s1], kq_d[h][:, 1, sm2:s1])
            if cb0 > 0:
                nc.gpsimd.dma_start(t["v"][:, 0:cb0], vh_d[h][:, 0:cb0])
            if cb0 + nch0 < NCH:
                nc.gpsimd.dma_start(t["v"][:, cb0 + nch0 :], vh_d[h][:, cb0 + nch0 :])
            for sl in rest:
                nc.sync.dma_start(t["kq"][:, :, sl], kq_d[h][:, :, sl])
            tiles[h] = t

        # ---- software-pipelined main loop (across jobs AND heads) ----
        # pending AV chunks are flushed lazily: up to AV_LAG newer QK chunks
        # issue first, so the PE is never idle waiting for the last exp of a
        # job.  Each pending entry carries its job context (ot tiles, flags)
        # and, for the final chunk of a job, the epilogue closure.
        # A job covers up to QN q columns in 512-col PIECES: its st tile and
        # one exp instruction span all pieces; QK/AV/copy go per piece
        # (PSUM-bank and accumulation-group granularity).
        pending = []  # (pt, j, pn, ctx, is_last_of_job)
        ep_state = {h: [None, 0, 0] for h in range(HPC)}  # [ep, cols, off0]

        out_q = [0]  # alternate output DMAs across the SP and GpSimd queues

        def ep_flush(h):
            est = ep_state[h]
            if est[0] is not None:
                eng = nc.sync if out_q[0] % 2 == 0 else nc.gpsimd
                out_q[0] += 1
                eng.dma_start(
                    out_d[:, h * S + est[2] : h * S + est[2] + est[1]],
                    est[0][:, 0 : est[1]],
                )
                est[0] = None

        def do_epilogue(ctx):
            h, qn, out_off = ctx["h"], ctx["qn"], ctx["out_off"]
            est = ep_state[h]
            if est[0] is not None and est[1] + qn > 1024:
                ep_flush(h)
            if est[0] is None:
                est[0] = epool.tile([D + 1, 1024], f32, name="ep", tag="ep")
                est[1] = 0
                est[2] = out_off
            ep, c0 = est[0], est[1]

            for (po, pqn), ot in zip(ctx["pieces"], ctx["ots"]):

                def act_cp(ot=ot, po=po, pqn=pqn):
                    nc.scalar.copy(ep[:, c0 + po : c0 + po + pqn], ot[:, 0:pqn])

                def dve_cp(ot=ot, po=po, pqn=pqn):
                    nc.vector.tensor_copy(ep[:, c0 + po : c0 + po + pqn], ot[:, 0:pqn])

                ops = {
                    "act": (ACT_COL, ACT_FIX, act_cp),
                    "dve": (DVE_COL, DVE_FIX, dve_cp),
                }
                balance(pqn, ops, ["act", "dve"])
            est[1] += qn

            if est[1] >= 1024 or out_off + qn >= S:  # tile full or last job
                ep_flush(h)

        # Epilogue copies are deferred one extra chunk so an ACT/DVE queue
        # never blocks at its head on a not-yet-finished AV while an exp
        # (which the PE needs sooner) sits behind it.
        epi_q = []

        def flush_pending(n_keep=0):
            while epi_q:
                do_epilogue(epi_q.pop(0))
            while len(pending) > n_keep:
                pt_, j_, pn_, ctx, last_ = pending.pop(0)
                for pi, (po, pqn) in enumerate(ctx["pieces"]):
                    ctx["n_av"][pi] += 1
                    nc.tensor.matmul(
                        ctx["ots"][pi][:, 0:pqn],
                        lhsT=ctx["v_sb"][0:pn_, ctx["cb"] + j_, :],
                        rhs=pt_[0:pn_, po : po + pqn],
                        start=ctx["n_av"][pi] == 1,
                        stop=last_,
                    )
                if last_:
                    epi_q.append(ctx)

        for h in range(HPC):
            kq_sb, v_sb = tiles[h]["kq"], tiles[h]["v"]

            for k0, L, cb, nch, qg, qn, out_off in jobs:
                pieces = [(po, min(512, qn - po)) for po in range(0, qn, 512)]
                ots = [
                    otpool.tile([D + 1, 512], f32, name="ot", tag="ot")
                    for _ in pieces
                ]
                ctx = {
                    "h": h, "ots": ots, "pieces": pieces, "qn": qn,
                    "out_off": out_off, "v_sb": v_sb, "cb": cb,
                    "n_av": [0] * len(pieces),
                }

                for j in range(nch):
                    pn = min(P, L - j * P)
                    st = stpool.tile([P, STW], f32, name="st", tag="st")
                    for po, pqn in pieces:
                        nc.tensor.matmul(
                            st[0:pn, po : po + pqn],
                            lhsT=kq_sb[:, 0, k0 + j * P : k0 + j * P + pn],
                            rhs=kq_sb[:, 1, qg + po : qg + po + pqn],
                            start=True,
                            stop=True,
                        )
                    pt = ptpool.tile([P, STW], bf16, name="pt", tag="pt")
                    schraud_ok = DVE_EXP and L >= DVE_MIN_L

                    def act_exp(pt=pt, st=st, pn=pn, qn=qn):
                        nc.scalar.activation(pt[0:pn, 0:qn], st[0:pn, 0:qn], EXP)

                    def dve_exp(pt=pt, st=st, pn=pn, qn=qn):
                        nc.vector.tensor_scalar(
                            pt[0:pn, 0:qn].bitcast(i16),
                            st[0:pn, 0:qn],
                            SCHRAUD_A,
                            SCHRAUD_B,
                            MUL,
                            ADD,
                        )

                    ops = {
                        "act": (ACT_COL, ACT_FIX, act_exp),
                        "dve": (DVE_COL, DVE_FIX, dve_exp),
                    }
                    balance(qn, ops, ["act", "dve"] if schraud_ok else ["act"])
                    flush_pending(n_keep=AV_LAG - 1)
                    pending.append((pt, j, pn, ctx, j == nch - 1))
        flush_pending()
        while epi_q:
            do_epilogue(epi_q.pop(0))

    nc.compile()
    return nc


def kernel(query_states, key_states, value_states, cu_seqlens, scaling):
    global LAST_RESULTS
    import ml_dtypes
    from concourse.bass_utils import run_bass_kernel_spmd

    q = np.asarray(query_states, dtype=np.float32)
    k = np.asarray(key_states, dtype=np.float32)
    v = np.asarray(value_states, dtype=np.float32)
    cu = np.asarray(cu_seqlens).astype(np.int64)
    sc = float(np.asarray(scaling))

    key = (tuple(int(x) for x in cu), DVE_EXP, POOL_EXP, WARMUP, QN, AV_LAG)
    nc = _nc_cache.get(key)
    if nc is None:
        nc = _nc_cache[key] = _build_nc(key[0])

    segs, NCH = _segments(cu)
    seg_order, jobs = _plan(segs)

    in_maps = []
    for c in range(N_CORES):
        hs = slice(c * HPC, (c + 1) * HPC)
        qt = (q[0, hs].transpose(0, 2, 1) * np.float32(sc)).astype(ml_dtypes.bfloat16)
        kt = k[0, hs].transpose(0, 2, 1).astype(ml_dtypes.bfloat16)
        kq = np.stack([kt, qt], axis=2)  # [HPC, D, 2, S]
        vp = np.zeros((HPC, P, NCH, D + 1), dtype=np.float32)
        for k0, L, cb, nch in segs:
            for j in range(nch):
                r0 = k0 + j * P
                pe = min(P, k0 + L - r0)
                vp[:, 0:pe, cb + j, 0:D] = v[0, hs, r0 : r0 + pe, :]
                vp[:, 0:pe, cb + j, D] = 1.0
        m = {
            "kq": np.ascontiguousarray(kq),
            "vh": vp.astype(ml_dtypes.bfloat16),
        }
        in_maps.append(m)

    LAST_RESULTS = run_bass_kernel_spmd(nc, in_maps, core_ids=list(range(N_CORES)))

    # host-side gather: divide numerators by the denominator row, undo the
    # job-major layout, and transpose each head's slab into [S, D]
    out = np.empty((1, S, H, D), dtype=np.float32)
    for c in range(N_CORES):
        slab = LAST_RESULTS.results[c]["out"]  # [D+1, HPC*S] job-major
        for h in range(HPC):
            o = slab[:, h * S : (h + 1) * S]
            d = o[0:D] / o[D : D + 1]  # [D, S] in job-major column order
            for k0, L, cb, nch, qg, qn, out_off in jobs:
                out[0, qg : qg + qn, c * HPC + h, :] = d[:, out_off : out_off + qn].T
    return out


# revision 43
# speedup vs baseline: 1.0153x; 1.0153x over previous
# Block-diagonal masked SDPA (Qwen2.5-VL vision style) for Trainium2.
#
# Full inputs:  q/k/v [1, 16, 4096, 80] f32, cu_seqlens [9] i32, scaling f32.
# Output:       [1, 4096, 16, 80] f32.
#
# Sharding: tensor-parallel over heads — 2 heads per core on 8 cores; each
# core computes its heads' full masked SDPA independently (no collectives).
#
# Strategy (host-specialized on cu_seqlens, same program on all cores):
#   Work is decomposed per SEGMENT, with k-chunks of 128 keys aligned to the
#   segment start, so no mask is ever needed: the last chunk of a segment
#   simply uses pn < 128 partitions.  V is host-packed segment-aligned as
#   [128, NCH, 81] bf16 (81st column = ones for the softmax denominator;
#   padding rows zero).  Everything runs as single bf16 matmuls: the 2e-2
#   harness gate leaves bf16 (~3e-3) ample margin.
#
#   Per segment, q is split into jobs of <= 512 columns.  Per chunk:
#     S^T [pn, qn] = K_chunk^T Q_job      (1 bf16 matmul, f32 PSUM)
#     P = exp(S^T) -> bf16 SBUF           (ACT exp, or DVE/Pool via a
#                                          Schraudolph bit-trick exp)
#     ot [81, qn] += V_chunk^T P          (1 bf16 matmul, V stationary)
#   Epilogue per job: evacuate ot PSUM->SBUF into half of a pair tile; one
#   DMA per job-PAIR ships the raw [81, .] numerator+denominator slab to a
#   job-major DRAM layout from the otherwise idle SP queue.  The gather
#   step on the host performs the final divide-by-denominator and [d, q] ->
#   [q, d] layout transpose.
#
# Trace-driven pipeline optimizations (v2):
#   - K and Q are host-packed into ONE DRAM tensor per head ([D, 2, S]);
#     input DMAs are issued K-of-first-segment first so the PE never waits,
#     and half as many DMA-issue instructions sit on the SP queue.
#   - The PE p-state ramps 0.65 -> 1.2 -> 2.4 GHz over ~3us of continuous
#     execution; WARMUP dummy matmuls during the DMA prologue pre-ramp the
#     clock so real matmuls start at speed.
#   - exp / copies are greedily balanced across THREE engines (ACT, DVE,
#     Pool); output DMAs moved off Pool (gpsimd) onto the idle SP queue.
#
# Exp instructions are widened (two chunks share one PSUM st tile and one
# exp) to amortize the ~200ns/instr access-latency bubble.  PSUM
# accumulation groups are bank-granular (2KB zero region): same-bank chunk
# pairs accumulate under one start/stop.

import os

import numpy as np

S = 4096
H = 16
D = 80
P = 128
N_CORES = 8
HPC = H // N_CORES  # heads per core

# Engine-balance cost model (ns) for ACT vs DVE vs Pool assignment.
ACT_COL = 1.0 / 1.2
ACT_FIX = 260.0
DVE_COL = 1.0 / 0.96
DVE_FIX = 155.0
POOL_COL = 1.0 / 0.5
POOL_FIX = 260.0

# Schraudolph exp on DVE/Pool: bf16(e^x) bit pattern ~= u16(x * 184.665 + B).
# +0.5 centers the f32->i16 truncation into round-to-nearest.
SCHRAUD_A = 128.0 / float(np.log(2.0))
SCHRAUD_B = 16250.5 + 0.5
# Only segments this long get Schraudolph exp: short segments have large
# softmax weights, amplifying the ~3% Schraudolph error in absolute terms.
DVE_MIN_L = 400

DVE_EXP = os.environ.get("KERNEL_DVE_EXP", "1") == "1"  # offload exp to DVE
# NOTE: GPSIMD/Pool cannot access PSUM (BIR verifier), so it cannot join the
# exp/copy balance (both read PSUM); it stays free for SBUF-side work only.
POOL_EXP = os.environ.get("KERNEL_POOL", "0") == "1"
WARMUP = int(os.environ.get("KERNEL_WARMUP", "8"))  # PE p-state pre-ramp
# AV groups stay pending while this many newer QK groups issue (possibly from
# the NEXT job/head), hiding the last-group exp latency at job boundaries.
AV_LAG = int(os.environ.get("KERNEL_AV_LAG", "3"))
# Job width in q columns.  1024 halves the exp instruction count (one exp
# per chunk covers the whole job width from a 2-bank st tile) — the per-
# instruction ~200-260ns ACT/DVE fixed cost is the binding resource once
# the matmul pipeline is clean.  512 gives narrower exps but 6 st tiles of
# runway instead of 3.
QN = int(os.environ.get("KERNEL_QN", "512"))
# Width of the very first job: narrower = smaller first DMA = earlier start.
FIRST_QN = int(os.environ.get("KERNEL_FIRST_QN", "512"))

_nc_cache = {}
LAST_RESULTS = None  # BassKernelResults of the most recent run (for test.py)


def _segments(cu):
    """[(k0, L, cb, nch)] per segment + total chunk count NCH."""
    segs = []
    cb = 0
    for s in range(len(cu) - 1):
        k0, k1 = int(cu[s]), int(cu[s + 1])
        L = k1 - k0
        if L == 0:
            continue
        nch = -(-L // P)
        segs.append((k0, L, cb, nch))
        cb += nch
    return segs, cb


def _plan(segs):
    """seg_order (largest first) and the global job list in processing order.

    jobs: [(k0, L, cb, nch, qg, qn, out_off)] — out_off is the column offset
    of this job's slab in the job-major DRAM output layout.

    Short segments (L < 512) are exp-bound (the PE finishes their few
    matmuls long before ACT/DVE finish their exps), so their jobs are
    interleaved between long-segment jobs instead of clustered at the end:
    the PE always has long-segment matmuls while a short job's exps drain.
    """
    seg_order = sorted(segs, key=lambda s: -s[1])
    big, small = [], []
    for idx, (k0, L, cb, nch) in enumerate(seg_order):
        o = 0
        while o < L:
            # the very first job stays narrow so it depends only on the
            # small first packed K+Q DMA
            cap = FIRST_QN if (idx == 0 and o == 0) else QN
            qn = min(cap, L - o)
            (big if L >= 512 else small).append((k0, L, cb, nch, k0 + o, qn))
            o += qn
    merged = big[:3]
    bi, si = 3, 0
    while bi < len(big):
        if si < len(small) - 1:  # keep the smallest job for the very end
            merged.append(small[si])
            si += 1
        merged.append(big[bi])
        bi += 1
    merged.extend(small[si:])
    if not big:
        merged = small
    jobs = []
    off = 0
    for k0, L, cb, nch, qg, qn in merged:
        jobs.append((k0, L, cb, nch, qg, qn, off))
        off += qn
    return seg_order, jobs


def _build_nc(cu_tuple):
    from contextlib import ExitStack

    import concourse.bass as bass  # noqa: F401
    import concourse.mybir as mybir
    import concourse.tile as tile
    from concourse import bacc

    f32 = mybir.dt.float32
    bf16 = mybir.dt.bfloat16
    i16 = mybir.dt.int16
    EXP = mybir.ActivationFunctionType.Exp
    MUL = mybir.AluOpType.mult
    ADD = mybir.AluOpType.add

    STW = 1024 if QN > 512 else 512
    ST_BUFS = 3 if QN > 512 else int(os.environ.get("KERNEL_ST_BUFS", "6"))
    OT_BUFS = 2 if QN > 512 else int(os.environ.get("KERNEL_OT_BUFS", "2"))
    cu = np.asarray(cu_tuple, dtype=np.int64)
    segs, NCH = _segments(cu)
    seg_order, jobs = _plan(segs)

    nc = bacc.Bacc(
        "TRN2",
        target_bir_lowering=False,
        debug=False,
        enable_asserts=False,
        num_devices=N_CORES,
    )

    # K at [:, 0, :], Q at [:, 1, :] (Q pre-scaled on host)
    kq_d = nc.dram_tensor("kq", [HPC, D, 2, S], bf16, kind="ExternalInput").ap()
    vh_d = nc.dram_tensor("vh", [HPC, P, NCH, D + 1], bf16, kind="ExternalInput").ap()
    # raw S^T-layout output slabs, job-major: numerators rows 0..79, denom 80
    out_d = nc.dram_tensor("out", [D + 1, HPC * S], f32, kind="ExternalOutput").ap()

    # Greedy ACT/DVE/Pool balance state (build-time, deterministic).
    t_eng = {"act": 0.0, "dve": 0.0, "pool": 0.0}

    def balance(cols, ops, allowed):
        """ops: {name: (col_cost, fix_cost, thunk)}; run on earliest-finish."""
        best, best_c = None, None
        for name in allowed:
            col_c, fix_c, op = ops[name]
            c = t_eng[name] + cols * col_c + fix_c
            if best_c is None or c < best_c:
                best, best_c = name, c
        t_eng[best] = best_c
        ops[best][2]()

    with ExitStack() as ctx:
        tc = ctx.enter_context(tile.TileContext(nc))
        io = ctx.enter_context(tc.tile_pool(name="io", bufs=2))
        wpool = ctx.enter_context(tc.tile_pool(name="wp", bufs=1))
        stpool = ctx.enter_context(
            tc.tile_pool(name="st", bufs=ST_BUFS, space="PSUM")
        )
        otpool = ctx.enter_context(tc.tile_pool(name="ot", bufs=OT_BUFS, space="PSUM"))
        ptpool = ctx.enter_context(tc.tile_pool(name="ptp", bufs=6))
        epool = ctx.enter_context(tc.tile_pool(name="ep", bufs=4))

        # --- PE p-state warmup: dummy matmuls during the DMA prologue ---
        # Reads uninitialized SBUF (harmless: output never read) so it has NO
        # dependencies and starts the instant the preamble barrier drops.
        # Rotates over 4 distinct PSUM regions so Tile's write-write tracking
        # doesn't serialize consecutive dummies.
        if WARMUP:
            wt = wpool.tile([P, 512], bf16, name="wt", tag="wt")
            nc.gpsimd.memset(wt[:], 0)
            wst = [
                stpool.tile([P, STW], f32, name="st", tag="st")
                for _ in range(2 if STW > 512 else 4)
            ]
            for i in range(WARMUP):
                if STW > 512:
                    w, half = wst[(i // 2) % 2], (i % 2) * 512
                else:
                    w, half = wst[i % 4], 0
                nc.tensor.matmul(
                    w[0:64, half : half + 512], lhsT=wt[:, 0:64], rhs=wt[:, 0:512],
                    start=True, stop=True,
                )

        # First processed segment's K loads first (full segment), then its
        # first q job, then V, then the rest — so the first QK matmuls of
        # every job are never waiting on DMA.
        s0 = seg_order[0][0]
        s1 = s0 + seg_order[0][1]
        qn1 = min(FIRST_QN, seg_order[0][1])
        rest = []
        if s0 > 0:
            rest.append(slice(0, s0))
        if s1 < S:
            rest.append(slice(s1, S))

        cb0, nch0 = seg_order[0][2], seg_order[0][3]
        sm1 = s0 + qn1  # first (narrow) job of the first segment
        sm2 = min(s0 + 1024, s1)

        tiles = {}
        for h in range(HPC):
            t = {}
            t["kq"] = io.tile([D, 2, S], bf16, name="kq_s", tag="kq")
            t["v"] = io.tile([P, NCH, D + 1], bf16, name="v_s", tag="v")
            # a small packed DMA covering exactly the first job's K and Q,
            # then a second packed slice (next jobs' Q + next K chunks),
            # then the rest — the first QK matmuls wait only on transfer 1
            # kq slices issue on the SP queue while ALL V slices issue in
            # parallel on the (otherwise idle) GpSimd queue — the first V
            # chunks land in time for the first job's AVs without waiting
            # behind the kq issue ladder.
            nc.sync.dma_start(t["kq"][:, :, s0:sm1], kq_d[h][:, :, s0:sm1])
            nc.gpsimd.dma_start(
                t["v"][:, cb0 : cb0 + nch0], vh_d[h][:, cb0 : cb0 + nch0]
            )
            if sm1 < sm2:
                nc.sync.dma_start(t["kq"][:, :, sm1:sm2], kq_d[h][:, :, sm1:sm2])
            if sm2 < s1:
                nc.sync.dma_start(t["kq"][:, 0, sm2:s1], kq_d[h][:, 0, sm2:s1])
            if sm2 < s1:
                nc.sync.dma_start(t["kq"][:, 1, sm2:s1], kq_d[h][:, 1, sm2:s1])
            if cb0 > 0:
                nc.gpsimd.dma_start(t["v"][:, 0:cb0], vh_d[h][:, 0:cb0])
            if cb0 + nch0 < NCH:
                nc.gpsimd.dma_start(t["v"][:, cb0 + nch0 :], vh_d[h][:, cb0 + nch0 :])
            for sl in rest:
                nc.sync.dma_start(t["kq"][:, :, sl], kq_d[h][:, :, sl])
            tiles[h] = t

        # ---- software-pipelined main loop (across jobs AND heads) ----
        # pending AV chunks are flushed lazily: up to AV_LAG newer QK chunks
        # issue first, so the PE is never idle waiting for the last exp of a
        # job.  Each pending entry carries its job context (ot tiles, flags)
        # and, for the final chunk of a job, the epilogue closure.
        # A job covers up to QN q columns in 512-col PIECES: its st tile and
        # one exp instruction span all pieces; QK/AV/copy go per piece
        # (PSUM-bank and accumulation-group granularity).
        pending = []  # (pt, j, pn, ctx, is_last_of_job)
        ep_state = {h: [None, 0, 0] for h in range(HPC)}  # [ep, cols, off0]

        def ep_flush(h):
            est = ep_state[h]
            if est[0] is not None:
                nc.sync.dma_start(
                    out_d[:, h * S + est[2] : h * S + est[2] + est[1]],
                    est[0][:, 0 : est[1]],
                )
                est[0] = None

        def do_epilogue(ctx):
            h, qn, out_off = ctx["h"], ctx["qn"], ctx["out_off"]
            est = ep_state[h]
            if est[0] is not None and est[1] + qn > 1024:
                ep_flush(h)
            if est[0] is None:
                est[0] = epool.tile([D + 1, 1024], f32, name="ep", tag="ep")
                est[1] = 0
                est[2] = out_off
            ep, c0 = est[0], est[1]

            for (po, pqn), ot in zip(ctx["pieces"], ctx["ots"]):

                def act_cp(ot=ot, po=po, pqn=pqn):
                    nc.scalar.copy(ep[:, c0 + po : c0 + po + pqn], ot[:, 0:pqn])

                def dve_cp(ot=ot, po=po, pqn=pqn):
                    nc.vector.tensor_copy(ep[:, c0 + po : c0 + po + pqn], ot[:, 0:pqn])

                ops = {
                    "act": (ACT_COL, ACT_FIX, act_cp),
                    "dve": (DVE_COL, DVE_FIX, dve_cp),
                }
                balance(pqn, ops, ["act", "dve"])
            est[1] += qn

            if est[1] >= 1024 or out_off + qn >= S:  # tile full or last job
                ep_flush(h)

        # Epilogue copies are deferred one extra chunk so an ACT/DVE queue
        # never blocks at its head on a not-yet-finished AV while an exp
        # (which the PE needs sooner) sits behind it.
        epi_q = []

        def flush_pending(n_keep=0):
            while epi_q:
                do_epilogue(epi_q.pop(0))
            while len(pending) > n_keep:
                pt_, j_, pn_, ctx, last_ = pending.pop(0)
                for pi, (po, pqn) in enumerate(ctx["pieces"]):
                    ctx["n_av"][pi] += 1
                    nc.tensor.matmul(
                        ctx["ots"][pi][:, 0:pqn],
                        lhsT=ctx["v_sb"][0:pn_, ctx["cb"] + j_, :],
                        rhs=pt_[0:pn_, po : po + pqn],
                        start=ctx["n_av"][pi] == 1,
                        stop=last_,
                    )
                if last_:
                    epi_q.append(ctx)

        for h in range(HPC):
            kq_sb, v_sb = tiles[h]["kq"], tiles[h]["v"]

            for k0, L, cb, nch, qg, qn, out_off in jobs:
                pieces = [(po, min(512, qn - po)) for po in range(0, qn, 512)]
                ots = [
                    otpool.tile([D + 1, 512], f32, name="ot", tag="ot")
                    for _ in pieces
                ]
                ctx = {
                    "h": h, "ots": ots, "pieces": pieces, "qn": qn,
                    "out_off": out_off, "v_sb": v_sb, "cb": cb,
                    "n_av": [0] * len(pieces),
                }

                for j in range(nch):
                    pn = min(P, L - j * P)
                    st = stpool.tile([P, STW], f32, name="st", tag="st")
                    for po, pqn in pieces:
                        nc.tensor.matmul(
                            st[0:pn, po : po + pqn],
                            lhsT=kq_sb[:, 0, k0 + j * P : k0 + j * P + pn],
                            rhs=kq_sb[:, 1, qg + po : qg + po + pqn],
                            start=True,
                            stop=True,
                        )
                    pt = ptpool.tile([P, STW], bf16, name="pt", tag="pt")
                    schraud_ok = DVE_EXP and L >= DVE_MIN_L

                    def act_exp(pt=pt, st=st, pn=pn, qn=qn):
                        nc.scalar.activation(pt[0:pn, 0:qn], st[0:pn, 0:qn], EXP)

                    def dve_exp(pt=pt, st=st, pn=pn, qn=qn):
                        nc.vector.tensor_scalar(
                            pt[0:pn, 0:qn].bitcast(i16),
                            st[0:pn, 0:qn],
                            SCHRAUD_A,
                            SCHRAUD_B,
                            MUL,
                            ADD,
                        )

                    ops = {
                        "act": (ACT_COL, ACT_FIX, act_exp),
                        "dve": (DVE_COL, DVE_FIX, dve_exp),
                    }
                    balance(qn, ops, ["act", "dve"] if schraud_ok else ["act"])
                    flush_pending(n_keep=AV_LAG - 1)
                    pending.append((pt, j, pn, ctx, j == nch - 1))
        flush_pending()
        while epi_q:
            do_epilogue(epi_q.pop(0))

    nc.compile()
    return nc


def kernel(query_states, key_states, value_states, cu_seqlens, scaling):
    global LAST_RESULTS
    import ml_dtypes
    from concourse.bass_utils import run_bass_kernel_spmd

    q = np.asarray(query_states, dtype=np.float32)
    k = np.asarray(key_states, dtype=np.float32)
    v = np.asarray(value_states, dtype=np.float32)
    cu = np.asarray(cu_seqlens).astype(np.int64)
    sc = float(np.asarray(scaling))

    key = (tuple(int(x) for x in cu), DVE_EXP, POOL_EXP, WARMUP, QN, AV_LAG)
    nc = _nc_cache.get(key)
    if nc is None:
        nc = _nc_cache[key] = _build_nc(key[0])

    segs, NCH = _segments(cu)
    seg_order, jobs = _plan(segs)

    in_maps = []
    for c in range(N_CORES):
        hs = slice(c * HPC, (c + 1) * HPC)
        qt = (q[0, hs].transpose(0, 2, 1) * np.float32(sc)).astype(ml_dtypes.bfloat16)
        kt = k[0, hs].transpose(0, 2, 1).astype(ml_dtypes.bfloat16)
        kq = np.stack([kt, qt], axis=2)  # [HPC, D, 2, S]
        vp = np.zeros((HPC, P, NCH, D + 1), dtype=np.float32)
        for k0, L, cb, nch in segs:
            for j in range(nch):
                r0 = k0 + j * P
                pe = min(P, k0 + L - r0)
                vp[:, 0:pe, cb + j, 0:D] = v[0, hs, r0 : r0 + pe, :]
                vp[:, 0:pe, cb + j, D] = 1.0
        m = {
            "kq": np.ascontiguousarray(kq),
            "vh": vp.astype(ml_dtypes.bfloat16),
        }
        in_maps.append(m)

    LAST_RESULTS = run_bass_kernel_spmd(nc, in_maps, core_ids=list(range(N_CORES)))

    # host-side gather: divide numerators by the denominator row, undo the
    # job-major layout, and transpose each head's slab into [S, D]
    out = np.empty((1, S, H, D), dtype=np.float32)
    for c in range(N_CORES):
        slab = LAST_RESULTS.results[c]["out"]  # [D+1, HPC*S] job-major
        for h in range(HPC):
            o = slab[:, h * S : (h + 1) * S]
            d = o[0:D] / o[D : D + 1]  # [D, S] in job-major column order
            for k0, L, cb, nch, qg, qn, out_off in jobs:
                out[0, qg : qg + qn, c * HPC + h, :] = d[:, out_off : out_off + qn].T
    return out
